# revision 22
# baseline (speedup 1.0000x reference)
"""Trainium2 Bass kernel for nn_AdaptiveAdjacencyMatrix.

Reference math:
    s[b, i]        = sum_d h[b, i, d] * w[d]
    scores[b,i,j]  = s[b,i] + s[b,j] + bias
    A              = softmax(scores, axis=1)   # over i

Because the softmax is over axis=1 (i), the `s[b,j] + bias` term is constant
along the reduced axis and cancels exactly:
    A[b, i, j] = exp(s[b,i]) / sum_i' exp(s[b,i'])   (independent of j and bias)

So the output is a column-broadcast of softmax(s[b]) — the kernel is purely
memory-bound. The output is written in bf16 (the host upcasts to f32), which
halves HBM write traffic vs f32; quantization error ~2^-9 is far inside the
accuracy budget.

Sharding: 8 cores = (batch b, row-half rh). Each core receives the full
h[b] (rows reordered so its own 2048 rows sit in the second half of the
buffer; that half is DMA'd FIRST so its dot products overlap the other
half's load), computes the full softmax sum locally (needs all 4096 rows;
row order is irrelevant to the sum), and writes a [2048, 4096] bf16 output
shard. No collectives.

Layouts: h is DMA'd with contiguous per-partition descriptors (partition p
holds rows 16p..16p+15 of a half, 4 KB runs per chunk). The output uses the
matching (q r) layout — device row q*16 + r holds the value for input row
16q + r — so the returned shard is already in natural row order (no host
permute) and each partition writes contiguous 8 KB HBM runs.

Measured shape (per NTFF traces): ~22 us startup — h reads cap at ~250
GB/s (two big-run DMAs; packet completions of concurrently queued DMAs
interleave in descriptor-arrival order, so fine chunking does NOT deliver
data earlier), dot/softmax on DVE+ACT overlaps the second half's load —
then the output stream saturates the per-core DMA system at ~412 GB/s for
40.7 us, ~5 us wind-down. Broadcast casts run on DVE (2 groups on ACT);
GpSimd measured ~7x slower at wide ops and stride-0 DMA source reads are
rejected by the compiler ("DGE fastest moving dim must be continuous"), so
tiles are materialized in SBUF. Slow runs (~80 us cluster) show all engine
ops dilated ~20% at constant DMA speed — device clock throttling, not a
kernel property.
"""

import ml_dtypes
import numpy as np

B, N, D = 4, 4096, 256
NCORES = 8
HALF = N // 2          # 2048 rows written per core
P = 128                # SBUF partitions
RPP = HALF // P        # 16 rows per partition (per half)
CH = 8                 # rows-per-partition per h-load/dot chunk (512 KB)
NG = RPP               # 16 output groups of [P, N] each (1 MB bf16)

_CACHE = {}


def _build():
    import concourse.mybir as mybir
    import concourse.tile as tile
    from concourse import bacc

    f32 = mybir.dt.float32
    bf16 = mybir.dt.bfloat16
    Copy = mybir.ActivationFunctionType.Copy
    Exp = mybir.ActivationFunctionType.Exp
    AX = mybir.AxisListType.X
    ADD = mybir.AluOpType.add
    MUL = mybir.AluOpType.mult
    nc = bacc.Bacc("TRN2", target_bir_lowering=False, debug=False)

    h_ext = nc.declare_dram_parameter("h", [N, D], bf16, isOutput=False)
    # w arrives pre-broadcast to [P, D] (tiny, lands first); it is repeated
    # to [P, CH, D] on DVE during the first h chunk's load so the multiplies
    # read real-strided bf16 (keeps DVE 2x mode).
    w_ext = nc.declare_dram_parameter("wb", [P, D], bf16, isOutput=False)
    out_ext = nc.declare_dram_parameter("out", [HALF, N], bf16, isOutput=True)

    # contiguous flat views: partition p holds rows 16p..16p+15 of each half
    h_oth = h_ext[0:HALF, :].rearrange("(p r) d -> p r d", p=P)
    h_mine = h_ext[HALF:N, :].rearrange("(p r) d -> p r d", p=P)
    # (q r) view of out: device row q*16 + r <-> e[q, r] (input row 16q + r),
    # so the shard comes back in natural order and partition q's writes are
    # contiguous in HBM.
    out_q = out_ext[:, :].rearrange("(q r) j -> q r j", r=RPP)

    with tile.TileContext(nc) as tc:
        with (
            tc.tile_pool(name="const", bufs=1) as cpool,
            tc.tile_pool(name="hload", bufs=2) as hpool,
            tc.tile_pool(name="prod", bufs=2) as ppool,
            tc.tile_pool(name="small", bufs=1) as spool,
            tc.tile_pool(name="obuf", bufs=8) as opool,
            tc.tile_pool(name="psum", bufs=1, space="PSUM") as psum_pool,
        ):
            # all-ones [128,128] for the PE cross-partition-sum trick
            ones_k = cpool.tile([P, P], f32)
            nc.vector.memset(ones_k[:, :], 1.0)

            # --- w (tiny, first on the sync ring), repeated on DVE while the
            # first h chunk streams in ---
            w_bc = cpool.tile([P, D], bf16)
            nc.sync.dma_start(out=w_bc[:, :], in_=w_ext[:, :])
            w_rep = cpool.tile([P, RPP, D], bf16)
            nc.vector.tensor_copy(
                w_rep[:, :, :],
                w_bc[:, :].unsqueeze(1).broadcast_to([P, RPP, D]),
            )

            # --- s = h @ w, one DMA per half on its own HWDGE ring.
            # Packet completions of concurrently queued DMAs interleave
            # round-robin across the 16 DMA engines, so fine-grained chunks
            # all complete at ~the same (late) time — two big DMAs with 8 KB
            # per-partition runs finish the whole load sooner (~408 GB/s vs
            # ~250 GB/s with 2-4 KB runs). Per half: one DVE multiply, a few
            # leading rows reduce on ACT (accum-reduce), the rest in one
            # batched DVE tensor_reduce — balances the two engines. Own half
            # first so e_mine is ready earliest. ---
            s_oth = spool.tile([P, RPP], f32)
            s_mine = spool.tile([P, RPP], f32)
            e_oth = spool.tile([P, RPP], f32)
            e_mine = spool.tile([P, RPP], f32)
            rs = spool.tile([P, 2], f32)
            jnk = spool.tile([P, D], f32)

            # (src, dma engine, s_dst, e_dst, act_rows, rs_col)
            halves = [
                (h_mine, nc.scalar, s_mine, e_mine, 5, 0),
                (h_oth, nc.sync, s_oth, e_oth, 4, 1),
            ]
            for h_src, h_dma_eng, s_dst, e_dst, act_rows, col in halves:
                hch = hpool.tile([P, RPP, D], bf16, tag="hch")
                h_dma_eng.dma_start(out=hch[:, :, :], in_=h_src[:, :, :])
                prod = ppool.tile([P, RPP, D], bf16, tag="prod")
                nc.vector.tensor_tensor(
                    out=prod[:, :, :],
                    in0=hch[:, :, :],
                    in1=w_rep[:, :, :],
                    op=MUL,
                )
                for g in range(act_rows):
                    nc.scalar.activation(
                        out=jnk[:, :],
                        in_=prod[:, g, :],
                        func=Copy,
                        accum_out=s_dst[:, g : g + 1],
                    )
                nc.vector.tensor_reduce(
                    out=s_dst[:, act_rows:RPP],
                    in_=prod[:, act_rows:RPP, :],
                    axis=AX,
                    op=ADD,
                )
                nc.scalar.activation(
                    out=e_dst[:, :],
                    in_=s_dst[:, :],
                    func=Exp,
                    accum_out=rs[:, col : col + 1],
                )

            # --- total sum: DVE column-reduce (also the single producer for
            # the PE), PE ones-matmul (sums partitions, broadcasts the result
            # to every partition), reciprocal straight from PSUM ---
            rs_sum = spool.tile([P, 1], f32)
            nc.vector.tensor_reduce(out=rs_sum[:, 0:1], in_=rs[:, 0:2], axis=AX, op=ADD)
            tot_psum = psum_pool.tile([P, 1], f32)
            nc.tensor.matmul(
                tot_psum[:, 0:1], ones_k[:, 0:P], rs_sum[:, 0:1], start=True, stop=True
            )
            inv = spool.tile([P, 1], f32)
            nc.vector.reciprocal(inv[:, 0:1], tot_psum[:, 0:1])

            # --- broadcast e/S along columns (stride-0 reads on e, the 1/S
            # multiply folded into the op) into bf16 tiles and stream out on
            # both HWDGE rings. One DMA per output group (fine interleave
            # keeps both queues evenly fed to the end); group 0 split into
            # four quarter-row DMAs for the earliest first byte. Two
            # mid-stream groups go on ACT (parallel feed while DVE casts);
            # GpSimd is useless here (measured ~7x slower than DVE and it
            # stalls concurrent DVE casts). ---
            def bcast(eng, dst, e_col):
                if eng == "act":
                    nc.scalar.activation(
                        out=dst, in_=e_col, func=Copy, scale=inv[:, 0:1]
                    )
                else:
                    nc.vector.tensor_scalar_mul(dst, e_col, inv[:, 0:1])

            ACT_GROUPS = (1, 4)
            nd = 0  # dma counter for queue alternation
            for g in range(NG):
                eng = "act" if g in ACT_GROUPS else "dve"
                ot = opool.tile([P, N], bf16, tag="ot")
                nq = 4 if g == 0 else 1
                for hj in range(nq):
                    j0, jw = hj * (N // nq), N // nq
                    bcast(
                        eng,
                        ot[:, j0 : j0 + jw],
                        e_mine[:, g : g + 1].broadcast_to([P, jw]),
                    )
                    dma_eng = nc.sync if nd % 2 == 0 else nc.scalar
                    nd += 1
                    dma_eng.dma_start(
                        out=out_q[:, g : g + 1, j0 : j0 + jw],
                        in_=ot[:, j0 : j0 + jw].rearrange("q (r j) -> q r j", r=1),
                    )
    nc.compile()
    return nc


def _get_nc():
    if "nc" not in _CACHE:
        _CACHE["nc"] = _build()
    return _CACHE["nc"]


def _ensure_axon_hooks():
    """bass_utils' trace path imports antenv.axon_hooks, which some images
    lack; provide a stub so tracing degrades instead of crashing. If the
    boot package + libaxon_pjrt.so are present, register the real
    ctypes-based NTFF profile hook so traced runs report exec_time_ns."""
    import sys
    import types

    try:
        import antenv.axon_hooks as m
    except ImportError:
        try:
            import antenv
        except ImportError:
            antenv = types.ModuleType("antenv")
            sys.modules["antenv"] = antenv
        m = types.ModuleType("antenv.axon_hooks")
        m._hook = None
        m.set_axon_ntff_profile_hook = lambda h: setattr(m, "_hook", h)
        m.get_axon_ntff_profile_hook = lambda: m._hook
        sys.modules["antenv.axon_hooks"] = m
    if m.get_axon_ntff_profile_hook() is None:
        try:
            import os

            from trn_agent_boot.trn_boot import _ntff_profile_via_ctypes

            so_path = "/opt/axon/libaxon_pjrt.so"
            if os.path.exists(so_path):
                hook = _ntff_profile_via_ctypes(so_path)
                if hook is not None:
                    m.set_axon_ntff_profile_hook(hook)
        except Exception:
            pass


def run_on_device(h, w, trace=False):
    """Run the SPMD kernel; returns the BassKernelResults."""
    from concourse.bass_utils import run_bass_kernel_spmd

    _ensure_axon_hooks()

    wb = np.ascontiguousarray(
        np.broadcast_to(w.astype(ml_dtypes.bfloat16), (P, D))
    )
    in_maps = []
    for c in range(NCORES):
        b_idx, rh = divmod(c, 2)
        hb = h[b_idx]
        # other half first (off the critical path), own half second
        if rh:
            hb_dev = hb
        else:
            hb_dev = np.concatenate([hb[HALF:], hb[:HALF]], axis=0)
        in_maps.append(
            {
                "h": np.ascontiguousarray(hb_dev.astype(ml_dtypes.bfloat16)),
                "wb": wb,
            }
        )
    res = run_bass_kernel_spmd(
        _get_nc(), in_maps, core_ids=list(range(NCORES)), trace=trace
    )
    return res


def kernel(h, w, b):
    h = np.asarray(h, dtype=np.float32)
    w = np.asarray(w, dtype=np.float32)
    res = run_on_device(h, w)
    A = np.empty((B, N, N), dtype=np.float32)
    for c in range(NCORES):
        b_idx, rh = divmod(c, 2)
        A[b_idx, rh * HALF : (rh + 1) * HALF, :] = res.results[c]["out"].astype(
            np.float32
        )
    return A
